# revision 15
# baseline (speedup 1.0000x reference)
"""BitNetLinear (ternary-quantized linear w/ training-blend) on 8 TRN2 NeuronCores.

Reference computation (fp32):
    thr  = mean(|W|)                       (global scalar over the full W)
    q    = sign(W) * (|W| > thr)           (ternary quantization)
    eff  = (1-l)*W + l*q, l=0.5            = 0.5*(W + q)
    eff  = eff * alpha
    out  = x @ eff^T + bias                x:[4,2048,4096] W:[4096,4096]

Sharding: tensor-parallel over out_features. Core c owns W rows
[c*512,(c+1)*512). x is replicated (pre-transposed to K-major bf16 on host),
the W shard is shipped K-major in fp32 (the threshold compare must see exact
fp32 values).

Two device phases (an on-device ncfw AllReduce measurably slows every
concurrent matmul ~20%, so the cross-core scalar reduction is done by
summing the 8 per-core partial outputs on the host instead — that sum is
just the unshard step of phase 1's reduce-scattered output):
  phase 1: each core reduces sum(|W_shard|) -> one fp32 scalar out.
  phase 2: takes the global sum as an input scalar; quantizes+blends the
    shard (fp32 math, bf16 effT cached in SBUF, [K,O] layout), streams
    x^T tiles, 2048 bf16 matmuls/core with fp32 PSUM accumulation, adds
    bias, writes the [8192, 512] fp32 output shard.
Host concatenates the 8 shards along the output-feature axis.
"""

import sys
import types

import numpy as np
import ml_dtypes


def _ensure_axon_hooks():
    """This image's antenv package lacks the axon_hooks submodule that
    concourse.bass_utils imports when tracing is requested (e.g. BASS_TRACE=1
    in the environment). Register a minimal stand-in so that path degrades
    gracefully instead of crashing."""
    try:
        import antenv.axon_hooks  # noqa: F401
        return
    except ImportError:
        pass
    try:
        import antenv
    except ImportError:
        return
    mod = types.ModuleType("antenv.axon_hooks")
    holder = {"hook": None}
    mod.set_axon_ntff_profile_hook = lambda h: holder.__setitem__("hook", h)
    mod.get_axon_ntff_profile_hook = lambda: holder["hook"]
    sys.modules["antenv.axon_hooks"] = mod
    antenv.axon_hooks = mod


_ensure_axon_hooks()

import concourse.bass as bass
import concourse.mybir as mybir
import concourse.tile as tile
from concourse import bacc
from concourse.bass_isa import ReduceOp
from concourse.bass_utils import run_bass_kernel_spmd

N_CORES = 8
CORE_IDS = list(range(N_CORES))

B, S, D_IN, D_OUT = 4, 2048, 4096, 4096
M = B * S                     # 8192 rows of x
O_SH = D_OUT // N_CORES       # 512 output features per core

P = 128                       # SBUF partitions
KO = D_IN // P                # 32 k-subtiles of 128
QCH = 4                       # k-subtiles per quantize chunk
NCH = KO // QCH               # 8 chunks
MT = 512                      # m-tile (x rows per output tile)
MS = MT // P                  # 4 PSUM subtiles per m-tile
NMT = M // MT                 # 16 m-tiles

_NC1 = None
_NC2 = None


def _build_phase1():
    """Per-core partial sum of |W_shard| -> [1,1] fp32.

    fp32 input: bf16 would halve the DMA but measures a systematic -2.2e-6
    relative bias on sum|w| (vs jnp's fp32 mean at ~3e-8), which moves the
    quantization threshold enough to flip ~35 mask elements and triple the
    absmax error. Not worth the ~5us.
    """
    dt = mybir.dt
    alu = mybir.AluOpType
    nc = bacc.Bacc("TRN2", target_bir_lowering=False, debug=False,
                   num_devices=N_CORES)
    wT = nc.dram_tensor("wT", [D_IN, O_SH], dt.float32, kind="ExternalInput").ap()
    psum_out = nc.dram_tensor("psum_out", [1, 1], dt.float32,
                              kind="ExternalOutput").ap()
    wT_r = wT.rearrange("(ko p) o -> p ko o", p=P)
    with tile.TileContext(nc) as tc:
        with (
            tc.tile_pool(name="persist", bufs=1) as persist,
            tc.tile_pool(name="wstage", bufs=4) as wstage,
        ):
            pp = persist.tile([P, KO], dt.float32)
            for g in range(NCH):
                wch = wstage.tile([P, QCH, O_SH], dt.float32, tag="wst",
                                  name=f"wch{g}")
                nc.sync.dma_start(wch[:], wT_r[:, g * QCH:(g + 1) * QCH, :])
                nc.vector.tensor_reduce(
                    pp[:, g * QCH:(g + 1) * QCH], wch[:],
                    axis=mybir.AxisListType.X, op=alu.add,
                    apply_absolute_value=True)
            part1 = persist.tile([P, 1], dt.float32)
            nc.vector.tensor_reduce(part1[:], pp[:], axis=mybir.AxisListType.X,
                                    op=alu.add)
            red = persist.tile([P, 1], dt.float32)
            nc.gpsimd.partition_all_reduce(red[:], part1[:], P, ReduceOp.add)
            nc.sync.dma_start(psum_out[:], red[0:1, :])
    nc.compile()
    return nc


def _build_phase2():
    dt = mybir.dt
    alu = mybir.AluOpType
    nc = bacc.Bacc("TRN2", target_bir_lowering=False, debug=False,
                   num_devices=N_CORES)

    xT = nc.dram_tensor("xT", [D_IN, M], dt.bfloat16, kind="ExternalInput").ap()
    wT = nc.dram_tensor("wT", [D_IN, O_SH], dt.float32, kind="ExternalInput").ap()
    bias_s = nc.dram_tensor("bias_s", [O_SH], dt.float32, kind="ExternalInput").ap()
    alpha_in = nc.dram_tensor("alpha_in", [1], dt.float32, kind="ExternalInput").ap()
    tot_in = nc.dram_tensor("tot_in", [1], dt.float32, kind="ExternalInput").ap()
    out = nc.dram_tensor("out", [M, O_SH], dt.float32, kind="ExternalOutput").ap()

    wT_r = wT.rearrange("(ko p) o -> p ko o", p=P)              # [128, 32, 512]
    xT_r = xT.rearrange("(ko p) m -> p ko m", p=P)              # [128, 32, 8192]
    out_r = out.rearrange("(mt ms p) o -> mt p ms o", p=P, ms=MS)

    with tile.TileContext(nc) as tc:
        with (
            tc.tile_pool(name="persist", bufs=1) as persist,
            tc.tile_pool(name="wstage", bufs=2) as wstage,
            tc.tile_pool(name="kxmp", bufs=3) as kxmp,
            tc.tile_pool(name="outp", bufs=3) as outp,
            tc.tile_pool(name="psum", bufs=2, space="PSUM") as psum,
        ):
            # ---- runtime scalars, broadcast per-partition ----
            # partition-broadcasts via K=1 PE matmul (ones[1,P].T @ row):
            # keeps GpSimd (and its slow library-reload) off the critical path
            # and gives the PE a head start.
            alpha_sb = persist.tile([1, 1], dt.float32)
            nc.sync.dma_start(alpha_sb[:], alpha_in[None, :])
            tot_sb = persist.tile([1, 1], dt.float32)
            nc.sync.dma_start(tot_sb[:], tot_in[None, :])
            sc_row = persist.tile([1, 4], dt.float32)
            nc.vector.memset(sc_row[:], 0.0)
            # sc_row = [c, thr, -thr, c/2]
            nc.vector.tensor_scalar_mul(sc_row[:, 0:1], alpha_sb[:], 0.5)
            nc.vector.tensor_scalar_mul(sc_row[:, 1:2], tot_sb[:],
                                        1.0 / (D_OUT * D_IN))
            nc.vector.tensor_scalar_mul(sc_row[:, 2:3], sc_row[:, 1:2], -1.0)
            nc.vector.tensor_scalar_mul(sc_row[:, 3:4], alpha_sb[:], 0.25)
            ones1 = persist.tile([1, P], dt.float32)
            nc.vector.memset(ones1[:], 1.0)
            psc = psum.tile([P, 4], dt.float32, tag="ps0", name="psc")
            nc.tensor.matmul(psc[:], ones1[:], sc_row[:], start=True, stop=True)
            sc_bc = persist.tile([P, 4], dt.float32)
            nc.vector.tensor_copy(sc_bc[:], psc[:])
            c_p = sc_bc[:, 0:1]
            thr_p = sc_bc[:, 1:2]
            negthr_p = sc_bc[:, 2:3]
            halfc_p = sc_bc[:, 3:4]

            bias_row = persist.tile([1, O_SH], dt.float32)
            nc.sync.dma_start(bias_row[:], bias_s[None, :])
            pbias = psum.tile([P, O_SH], dt.float32, tag="ps1", name="pbias")
            nc.tensor.matmul(pbias[:], ones1[:], bias_row[:], start=True,
                             stop=True)
            bias_bc = persist.tile([P, O_SH], dt.float32)
            nc.vector.tensor_copy(bias_bc[:], pbias[:])

            # ---- quantize + blend -> effT bf16 [K, O] cached in SBUF ----
            # eff = c*(w + q), q = (sign(w-thr) + sign(w+thr)) / 2
            # (equivalent to (w>thr)-(w<-thr) except at exact fp32 ties,
            # which have ~zero probability; w-thr is exact near the
            # threshold by Sterbenz). The two sign passes run on the
            # otherwise-idle Scalar engine so DVE only does the 3 combine
            # passes — quantize throughput paces the first m-tiles.
            effT = persist.tile([P, KO, O_SH], dt.bfloat16)
            # ladder: small first chunks so the first matmuls start early;
            # steady chunks of 4 k-subtiles once the PE stream is rolling
            chunks = [1, 1, 2] + [QCH] * ((KO - 4) // QCH)
            assert sum(chunks) == KO
            pos = 0
            for g, ch in enumerate(chunks):
                sl = slice(pos, pos + ch)
                pos += ch
                wch = wstage.tile([P, QCH, O_SH], dt.float32, tag="wst",
                                  name=f"wch{g}")[:, :ch, :]
                nc.sync.dma_start(wch[:], wT_r[:, sl, :])
                s1 = wstage.tile([P, QCH, O_SH], dt.float32, tag="s1",
                                 name=f"s1_{g}", bufs=2)[:, :ch, :]
                nc.scalar.activation(s1[:], wch[:],
                                     mybir.ActivationFunctionType.Sign,
                                     bias=negthr_p[:])
                s2 = wstage.tile([P, QCH, O_SH], dt.float32, tag="s2",
                                 name=f"s2_{g}", bufs=2)[:, :ch, :]
                nc.scalar.activation(s2[:], wch[:],
                                     mybir.ActivationFunctionType.Sign,
                                     bias=thr_p[:])
                # cw = c*w in place on the Scalar engine, then two fused
                # multiply-adds on DVE: eff = (c/2)*s1 + ((c/2)*s2 + c*w)
                nc.scalar.activation(wch[:], wch[:],
                                     mybir.ActivationFunctionType.Copy,
                                     scale=c_p[:])
                nc.vector.scalar_tensor_tensor(
                    out=s2[:], in0=s2[:], scalar=halfc_p[:], in1=wch[:],
                    op0=alu.mult, op1=alu.add)
                nc.vector.scalar_tensor_tensor(
                    out=effT[:, sl, :], in0=s1[:], scalar=halfc_p[:], in1=s2[:],
                    op0=alu.mult, op1=alu.add)

            # ---- main matmul stream: out[m, o] = sum_k x[m,k] * eff[o,k] ----
            # m-tiles 0,1 run ksub-major across all 8 PSUM banks so the PE
            # consumes effT chunks at the rate the DVE quantize produces them
            pair = (0, 1)
            kxms = {}
            for mt in pair:
                kxm = kxmp.tile([P, KO, MT], dt.bfloat16, tag="kxm",
                                name=f"kxm{mt}")
                msl = slice(mt * MT, (mt + 1) * MT)
                for g in range(NCH):
                    nc.sync.dma_start(
                        kxm[:, g * QCH:(g + 1) * QCH, :],
                        xT_r[:, g * QCH:(g + 1) * QCH, msl])
                kxms[mt] = kxm
            ppts = {mt: [psum.tile([P, O_SH], dt.float32, tag=f"ps{j}",
                                   name=f"ps{j}_{mt}") for j in range(MS)]
                    for mt in pair}
            for ko in range(KO):
                for mt in pair:
                    for j in range(MS):
                        nc.tensor.matmul(
                            ppts[mt][j][:],
                            kxms[mt][:, ko, j * P:(j + 1) * P],
                            effT[:, ko, :],
                            start=(ko == 0), stop=(ko == KO - 1))
            for mt in pair:
                ot = outp.tile([P, MS, O_SH], dt.float32, tag="ot",
                               name=f"ot{mt}")
                for j in range(MS):
                    nc.vector.tensor_tensor(ot[:, j, :], ppts[mt][j][:],
                                            bias_bc[:], alu.add)
                nc.sync.dma_start(out_r[mt], ot[:])

            for mt in range(2, NMT):
                kxm = kxmp.tile([P, KO, MT], dt.bfloat16, tag="kxm",
                                name=f"kxm{mt}")
                msl = slice(mt * MT, (mt + 1) * MT)
                for g in range(NCH):
                    nc.sync.dma_start(
                        kxm[:, g * QCH:(g + 1) * QCH, :],
                        xT_r[:, g * QCH:(g + 1) * QCH, msl])
                pts = [psum.tile([P, O_SH], dt.float32, tag=f"ps{j}",
                                 name=f"ps{j}_{mt}") for j in range(MS)]
                for ko in range(KO):
                    for j in range(MS):
                        nc.tensor.matmul(
                            pts[j][:],
                            kxm[:, ko, j * P:(j + 1) * P],
                            effT[:, ko, :],
                            start=(ko == 0), stop=(ko == KO - 1))
                ot = outp.tile([P, MS, O_SH], dt.float32, tag="ot",
                               name=f"ot{mt}")
                for j in range(MS):
                    nc.vector.tensor_tensor(ot[:, j, :], pts[j][:], bias_bc[:],
                                            alu.add)
                    if mt == NMT - 1:
                        # finer stores at the end shorten the kernel tail
                        nc.sync.dma_start(out_r[mt][:, j, :], ot[:, j, :])
                if mt != NMT - 1:
                    nc.sync.dma_start(out_r[mt], ot[:])

    nc.compile()
    return nc


def _get_ncs():
    global _NC1, _NC2
    if _NC1 is None:
        _NC1 = _build_phase1()
    if _NC2 is None:
        _NC2 = _build_phase2()
    return _NC1, _NC2


def kernel(x: np.ndarray, weight_fp: np.ndarray, bias: np.ndarray,
           alpha: np.ndarray, _trace: bool = False, **_kw):
    x = np.asarray(x)
    weight_fp = np.asarray(weight_fp, dtype=np.float32)
    bias = np.asarray(bias, dtype=np.float32)
    alpha = np.asarray(alpha, dtype=np.float32)

    # host-side layout prep: x -> K-major bf16 (replicated), W shard -> K-major fp32
    x2 = np.ascontiguousarray(
        x.reshape(M, D_IN).astype(ml_dtypes.bfloat16).T)       # [D_IN, M]
    wshards = [np.ascontiguousarray(weight_fp[c * O_SH:(c + 1) * O_SH, :].T)
               for c in range(N_CORES)]                        # [D_IN, O_SH]

    nc1, nc2 = _get_ncs()

    # phase 1: per-core partial sums of |W|
    in1 = [{"wT": wshards[c]} for c in range(N_CORES)]
    res1 = run_bass_kernel_spmd(nc1, in1, CORE_IDS, trace=_trace)
    total = np.float32(sum(np.float64(res1.results[c]["psum_out"][0, 0])
                           for c in range(N_CORES)))

    # phase 2: quantize + matmul
    in2 = []
    for c in range(N_CORES):
        in2.append({
            "xT": x2,
            "wT": wshards[c],
            "bias_s": np.ascontiguousarray(bias[c * O_SH:(c + 1) * O_SH]),
            "alpha_in": alpha,
            "tot_in": np.array([total], dtype=np.float32),
        })
    res2 = run_bass_kernel_spmd(nc2, in2, CORE_IDS, trace=_trace)
    shards = [res2.results[c]["out"] for c in range(N_CORES)]
    full = np.concatenate(shards, axis=1).reshape(B, S, D_OUT)
    if _trace:
        kernel.last_exec_time_ns = (res1.exec_time_ns or 0) + (res2.exec_time_ns or 0)
        kernel.last_phase_times = (res1.exec_time_ns, res2.exec_time_ns)
    return full


if __name__ == "__main__":
    rng = np.random.default_rng(0)
    x = rng.standard_normal((B, S, D_IN), dtype=np.float32)
    w = rng.standard_normal((D_OUT, D_IN), dtype=np.float32)
    b = np.zeros(D_OUT, np.float32)
    a = np.ones(1, np.float32)
    out = kernel(x, w, b, a)
    print("out", out.shape, out.dtype, out[0, 0, :4])


# revision 16
# speedup vs baseline: 1.1896x; 1.1896x over previous
"""BitNetLinear (ternary-quantized linear w/ training-blend) on 8 TRN2 NeuronCores.

Reference computation (fp32):
    thr  = mean(|W|)                       (global scalar over the full W)
    q    = sign(W) * (|W| > thr)           (ternary quantization)
    eff  = (1-l)*W + l*q, l=0.5            = 0.5*(W + q)
    eff  = eff * alpha
    out  = x @ eff^T + bias                x:[4,2048,4096] W:[4096,4096]

Sharding: tensor-parallel over out_features. Core c owns W rows
[c*512,(c+1)*512). x is replicated (pre-transposed to K-major bf16 on host),
the W shard is shipped K-major in fp32 (the threshold compare must see exact
fp32 values).

Two device phases (an on-device ncfw AllReduce measurably slows every
concurrent matmul ~20%, so the cross-core scalar reduction is done by
summing the 8 per-core partial outputs on the host instead — that sum is
just the unshard step of phase 1's reduce-scattered output):
  phase 1: each core reduces sum(|W_shard|) -> one fp32 scalar out.
  phase 2: takes the global sum as an input scalar; quantizes+blends the
    shard (fp32 math, bf16 effT cached in SBUF, [K,O] layout), streams
    x^T tiles, 2048 bf16 matmuls/core with fp32 PSUM accumulation, adds
    bias, writes the [8192, 512] fp32 output shard.
Host concatenates the 8 shards along the output-feature axis.
"""

import sys
import types

import numpy as np
import ml_dtypes


def _ensure_axon_hooks():
    """This image's antenv package lacks the axon_hooks submodule that
    concourse.bass_utils imports when tracing is requested (e.g. BASS_TRACE=1
    in the environment). Register a minimal stand-in so that path degrades
    gracefully instead of crashing."""
    try:
        import antenv.axon_hooks  # noqa: F401
        return
    except ImportError:
        pass
    try:
        import antenv
    except ImportError:
        return
    mod = types.ModuleType("antenv.axon_hooks")
    holder = {"hook": None}
    mod.set_axon_ntff_profile_hook = lambda h: holder.__setitem__("hook", h)
    mod.get_axon_ntff_profile_hook = lambda: holder["hook"]
    sys.modules["antenv.axon_hooks"] = mod
    antenv.axon_hooks = mod


_ensure_axon_hooks()

import concourse.bass as bass
import concourse.mybir as mybir
import concourse.tile as tile
from concourse import bacc
from concourse.bass_isa import ReduceOp
from concourse.bass_utils import run_bass_kernel_spmd

N_CORES = 8
CORE_IDS = list(range(N_CORES))

B, S, D_IN, D_OUT = 4, 2048, 4096, 4096
M = B * S                     # 8192 rows of x
O_SH = D_OUT // N_CORES       # 512 output features per core

P = 128                       # SBUF partitions
KO = D_IN // P                # 32 k-subtiles of 128
QCH = 4                       # k-subtiles per quantize chunk
NCH = KO // QCH               # 8 chunks
MT = 512                      # m-tile (x rows per output tile)
MS = MT // P                  # 4 PSUM subtiles per m-tile
NMT = M // MT                 # 16 m-tiles

_NC1 = None
_NC2 = None


def _build_phase1():
    """Per-core partial sum of |W_shard| -> [1,1] fp32.

    fp32 input: bf16 would halve the DMA but measures a systematic -2.2e-6
    relative bias on sum|w| (vs jnp's fp32 mean at ~3e-8), which moves the
    quantization threshold enough to flip ~35 mask elements and triple the
    absmax error. Not worth the ~5us.
    """
    dt = mybir.dt
    alu = mybir.AluOpType
    nc = bacc.Bacc("TRN2", target_bir_lowering=False, debug=False,
                   num_devices=N_CORES)
    wT = nc.dram_tensor("wT", [D_IN, O_SH], dt.float32, kind="ExternalInput").ap()
    psum_out = nc.dram_tensor("psum_out", [1, 1], dt.float32,
                              kind="ExternalOutput").ap()
    wT_r = wT.rearrange("(ko p) o -> p ko o", p=P)
    with tile.TileContext(nc) as tc:
        with (
            tc.tile_pool(name="persist", bufs=1) as persist,
            tc.tile_pool(name="wstage", bufs=4) as wstage,
        ):
            pp = persist.tile([P, KO], dt.float32)
            for g in range(NCH):
                wch = wstage.tile([P, QCH, O_SH], dt.float32, tag="wst",
                                  name=f"wch{g}")
                nc.sync.dma_start(wch[:], wT_r[:, g * QCH:(g + 1) * QCH, :])
                nc.vector.tensor_reduce(
                    pp[:, g * QCH:(g + 1) * QCH], wch[:],
                    axis=mybir.AxisListType.X, op=alu.add,
                    apply_absolute_value=True)
            part1 = persist.tile([P, 1], dt.float32)
            nc.vector.tensor_reduce(part1[:], pp[:], axis=mybir.AxisListType.X,
                                    op=alu.add)
            red = persist.tile([P, 1], dt.float32)
            nc.gpsimd.partition_all_reduce(red[:], part1[:], P, ReduceOp.add)
            nc.sync.dma_start(psum_out[:], red[0:1, :])
    nc.compile()
    return nc


def _build_phase2():
    dt = mybir.dt
    alu = mybir.AluOpType
    nc = bacc.Bacc("TRN2", target_bir_lowering=False, debug=False,
                   num_devices=N_CORES)

    xT = nc.dram_tensor("xT", [D_IN, M], dt.bfloat16, kind="ExternalInput").ap()
    wT = nc.dram_tensor("wT", [D_IN, O_SH], dt.float32, kind="ExternalInput").ap()
    bias_s = nc.dram_tensor("bias_s", [O_SH], dt.float32, kind="ExternalInput").ap()
    alpha_in = nc.dram_tensor("alpha_in", [1], dt.float32, kind="ExternalInput").ap()
    tot_in = nc.dram_tensor("tot_in", [1], dt.float32, kind="ExternalInput").ap()
    out = nc.dram_tensor("out", [M, O_SH], dt.float32, kind="ExternalOutput").ap()

    wT_r = wT.rearrange("(ko p) o -> p ko o", p=P)              # [128, 32, 512]
    xT_r = xT.rearrange("(ko p) m -> p ko m", p=P)              # [128, 32, 8192]
    out_r = out.rearrange("(mt ms p) o -> mt p ms o", p=P, ms=MS)

    with tile.TileContext(nc) as tc:
        with (
            tc.tile_pool(name="persist", bufs=1) as persist,
            tc.tile_pool(name="wstage", bufs=2) as wstage,
            tc.tile_pool(name="kxmp", bufs=3) as kxmp,
            tc.tile_pool(name="outp", bufs=3) as outp,
            tc.tile_pool(name="psum", bufs=2, space="PSUM") as psum,
        ):
            # ---- runtime scalars, broadcast per-partition ----
            # partition-broadcasts via K=1 PE matmul (ones[1,P].T @ row):
            # keeps GpSimd (and its slow library-reload) off the critical path
            # and gives the PE a head start.
            alpha_sb = persist.tile([1, 1], dt.float32)
            nc.sync.dma_start(alpha_sb[:], alpha_in[None, :])
            tot_sb = persist.tile([1, 1], dt.float32)
            nc.sync.dma_start(tot_sb[:], tot_in[None, :])
            sc_row = persist.tile([1, 4], dt.float32)
            nc.vector.memset(sc_row[:], 0.0)
            # sc_row = [c, thr, -thr, 0]
            nc.vector.tensor_scalar_mul(sc_row[:, 0:1], alpha_sb[:], 0.5)
            nc.vector.tensor_scalar_mul(sc_row[:, 1:2], tot_sb[:],
                                        1.0 / (D_OUT * D_IN))
            nc.vector.tensor_scalar_mul(sc_row[:, 2:3], sc_row[:, 1:2], -1.0)
            ones1 = persist.tile([1, P], dt.float32)
            nc.vector.memset(ones1[:], 1.0)
            psc = psum.tile([P, 4], dt.float32, tag="ps0", name="psc")
            nc.tensor.matmul(psc[:], ones1[:], sc_row[:], start=True, stop=True)
            sc_bc = persist.tile([P, 4], dt.float32)
            nc.vector.tensor_copy(sc_bc[:], psc[:])
            c_p = sc_bc[:, 0:1]
            thr_p = sc_bc[:, 1:2]
            negthr_p = sc_bc[:, 2:3]

            bias_row = persist.tile([1, O_SH], dt.float32)
            nc.sync.dma_start(bias_row[:], bias_s[None, :])
            pbias = psum.tile([P, O_SH], dt.float32, tag="ps1", name="pbias")
            nc.tensor.matmul(pbias[:], ones1[:], bias_row[:], start=True,
                             stop=True)
            bias_bc = persist.tile([P, O_SH], dt.float32)
            nc.vector.tensor_copy(bias_bc[:], pbias[:])

            # ---- quantize + blend -> effT bf16 [K, O] cached in SBUF ----
            # eff = c*(w + q), q = (sign(w-thr) + sign(w+thr)) / 2
            # (equivalent to (w>thr)-(w<-thr) except at exact fp32 ties,
            # which have ~zero probability; w-thr is exact near the
            # threshold by Sterbenz). The two sign passes run on the
            # otherwise-idle Scalar engine so DVE only does the 3 combine
            # passes — quantize throughput paces the first m-tiles.
            effT = persist.tile([P, KO, O_SH], dt.bfloat16)
            # ladder: small first chunks so the first matmuls start early;
            # steady chunks of 4 k-subtiles once the PE stream is rolling
            chunks = [1, 1, 2] + [QCH] * ((KO - 4) // QCH)
            assert sum(chunks) == KO
            pos = 0
            for g, ch in enumerate(chunks):
                sl = slice(pos, pos + ch)
                pos += ch
                wch = wstage.tile([P, QCH, O_SH], dt.float32, tag="wst",
                                  name=f"wch{g}")[:, :ch, :]
                nc.sync.dma_start(wch[:], wT_r[:, sl, :])
                s1 = wstage.tile([P, QCH, O_SH], dt.float32, tag="s1",
                                 name=f"s1_{g}", bufs=2)[:, :ch, :]
                nc.scalar.activation(s1[:], wch[:],
                                     mybir.ActivationFunctionType.Sign,
                                     bias=negthr_p[:])
                s2 = wstage.tile([P, QCH, O_SH], dt.float32, tag="s2",
                                 name=f"s2_{g}", bufs=2)[:, :ch, :]
                nc.scalar.activation(s2[:], wch[:],
                                     mybir.ActivationFunctionType.Sign,
                                     bias=thr_p[:])
                nc.vector.tensor_tensor(s1[:], s1[:], s2[:], alu.add)
                nc.vector.scalar_tensor_tensor(
                    out=s2[:], in0=s1[:], scalar=0.5, in1=wch[:],
                    op0=alu.mult, op1=alu.add)
                nc.vector.tensor_scalar_mul(effT[:, sl, :], s2[:], c_p[:])

            # ---- main matmul stream: out[m, o] = sum_k x[m,k] * eff[o,k] ----
            # m-tiles 0,1 run ksub-major across all 8 PSUM banks so the PE
            # consumes effT chunks at the rate the DVE quantize produces them
            pair = (0, 1)
            kxms = {}
            for mt in pair:
                kxm = kxmp.tile([P, KO, MT], dt.bfloat16, tag="kxm",
                                name=f"kxm{mt}")
                msl = slice(mt * MT, (mt + 1) * MT)
                for g in range(NCH):
                    nc.sync.dma_start(
                        kxm[:, g * QCH:(g + 1) * QCH, :],
                        xT_r[:, g * QCH:(g + 1) * QCH, msl])
                kxms[mt] = kxm
            ppts = {mt: [psum.tile([P, O_SH], dt.float32, tag=f"ps{j}",
                                   name=f"ps{j}_{mt}") for j in range(MS)]
                    for mt in pair}
            for ko in range(KO):
                for mt in pair:
                    for j in range(MS):
                        nc.tensor.matmul(
                            ppts[mt][j][:],
                            kxms[mt][:, ko, j * P:(j + 1) * P],
                            effT[:, ko, :],
                            start=(ko == 0), stop=(ko == KO - 1))
            for mt in pair:
                ot = outp.tile([P, MS, O_SH], dt.float32, tag="ot",
                               name=f"ot{mt}")
                for j in range(MS):
                    nc.vector.tensor_tensor(ot[:, j, :], ppts[mt][j][:],
                                            bias_bc[:], alu.add)
                nc.sync.dma_start(out_r[mt], ot[:])

            for mt in range(2, NMT):
                kxm = kxmp.tile([P, KO, MT], dt.bfloat16, tag="kxm",
                                name=f"kxm{mt}")
                msl = slice(mt * MT, (mt + 1) * MT)
                for g in range(NCH):
                    nc.sync.dma_start(
                        kxm[:, g * QCH:(g + 1) * QCH, :],
                        xT_r[:, g * QCH:(g + 1) * QCH, msl])
                pts = [psum.tile([P, O_SH], dt.float32, tag=f"ps{j}",
                                 name=f"ps{j}_{mt}") for j in range(MS)]
                for ko in range(KO):
                    for j in range(MS):
                        nc.tensor.matmul(
                            pts[j][:],
                            kxm[:, ko, j * P:(j + 1) * P],
                            effT[:, ko, :],
                            start=(ko == 0), stop=(ko == KO - 1))
                ot = outp.tile([P, MS, O_SH], dt.float32, tag="ot",
                               name=f"ot{mt}")
                for j in range(MS):
                    nc.vector.tensor_tensor(ot[:, j, :], pts[j][:], bias_bc[:],
                                            alu.add)
                    if mt == NMT - 1:
                        # finer stores at the end shorten the kernel tail
                        nc.sync.dma_start(out_r[mt][:, j, :], ot[:, j, :])
                if mt != NMT - 1:
                    nc.sync.dma_start(out_r[mt], ot[:])

    nc.compile()
    return nc


def _get_ncs():
    global _NC1, _NC2
    if _NC1 is None:
        _NC1 = _build_phase1()
    if _NC2 is None:
        _NC2 = _build_phase2()
    return _NC1, _NC2


def kernel(x: np.ndarray, weight_fp: np.ndarray, bias: np.ndarray,
           alpha: np.ndarray, _trace: bool = False, **_kw):
    x = np.asarray(x)
    weight_fp = np.asarray(weight_fp, dtype=np.float32)
    bias = np.asarray(bias, dtype=np.float32)
    alpha = np.asarray(alpha, dtype=np.float32)

    # host-side layout prep: x -> K-major bf16 (replicated), W shard -> K-major fp32
    x2 = np.ascontiguousarray(
        x.reshape(M, D_IN).astype(ml_dtypes.bfloat16).T)       # [D_IN, M]
    wshards = [np.ascontiguousarray(weight_fp[c * O_SH:(c + 1) * O_SH, :].T)
               for c in range(N_CORES)]                        # [D_IN, O_SH]

    nc1, nc2 = _get_ncs()

    # phase 1: per-core partial sums of |W|
    in1 = [{"wT": wshards[c]} for c in range(N_CORES)]
    res1 = run_bass_kernel_spmd(nc1, in1, CORE_IDS, trace=_trace)
    total = np.float32(sum(np.float64(res1.results[c]["psum_out"][0, 0])
                           for c in range(N_CORES)))

    # phase 2: quantize + matmul
    in2 = []
    for c in range(N_CORES):
        in2.append({
            "xT": x2,
            "wT": wshards[c],
            "bias_s": np.ascontiguousarray(bias[c * O_SH:(c + 1) * O_SH]),
            "alpha_in": alpha,
            "tot_in": np.array([total], dtype=np.float32),
        })
    res2 = run_bass_kernel_spmd(nc2, in2, CORE_IDS, trace=_trace)
    shards = [res2.results[c]["out"] for c in range(N_CORES)]
    full = np.concatenate(shards, axis=1).reshape(B, S, D_OUT)
    if _trace:
        kernel.last_exec_time_ns = (res1.exec_time_ns or 0) + (res2.exec_time_ns or 0)
        kernel.last_phase_times = (res1.exec_time_ns, res2.exec_time_ns)
    return full


if __name__ == "__main__":
    rng = np.random.default_rng(0)
    x = rng.standard_normal((B, S, D_IN), dtype=np.float32)
    w = rng.standard_normal((D_OUT, D_IN), dtype=np.float32)
    b = np.zeros(D_OUT, np.float32)
    a = np.ones(1, np.float32)
    out = kernel(x, w, b, a)
    print("out", out.shape, out.dtype, out[0, 0, :4])
